# revision 7
# baseline (speedup 1.0000x reference)
"""DistMult decoder kernel for 8 Trainium2 NeuronCores.

Computes out = (input1 * weight[type_index]) @ input2.T + bias with
input1 [8192, 512], input2 [8192, 512] in fp32, out [8192, 8192].

Sharding: rows of input1 (and thus rows of the output) are split across
the 8 cores; input2 / weight / bias are replicated. No communication.

Per-core device program (M = 1024 rows):
  - lhsT  [MT, P, KT*128] = w_r-scaled shard of input1, packed on host
    into per-m-tile blocks (one contiguous 128 KB DMA per m-tile)
  - rhs   [512, 8192]  = input2 transposed + cast to fp16 on host
  - fp16 operands run the PE at 1 cycle/row with fp32 PSUM accumulation
  - GEMM over n-pairs: 8 groups x 8 m-tiles x (4 k x 2 n) matmuls,
    215.8 ns/matmul steady state (the N=512 fp16 streaming floor)
  - output stored as fp16 (16 MB/core instead of 32 MB) and upcast on
    host; total quantization error ~8e-4 vs the 2e-2 gate

Head/tail schedule (from baseline trace analysis: body exec starts
~6.1 us after a fixed preamble, first DMA data lands ~3 us later):
  - 8 warmup matmuls on zeroed SBUF start at body-exec time and span
    the ~3.4 us HAM busy window, so the PE clock is at/near 2.4 GHz
    when the first real matmul's data arrives (~9.5 us)
  - head DMAs are priority-ordered across the three DGE rings: the
    first n-group quarters and first m-tiles land before later ones
  - last tile: ps0 copy overlaps ps1's matmuls, ps1 copy is split
    ACT/DVE, and the final stores fan out over all three rings
"""

import os

import numpy as np

import concourse.bacc as bacc
import concourse.mybir as mybir
from concourse.bass_utils import run_bass_kernel_spmd
from concourse.tile import TileContext

N_CORES = 8
N1, N2, D = 8192, 8192, 512
M = N1 // N_CORES  # rows per core
P = 128            # partitions
KT = D // P        # 4 k-tiles
MT = M // P        # 8 m-tiles
NFREE = 512        # psum bank free size (fp32)
NGRP = 1024        # n columns per group (pair of psum banks)
NT = N2 // NGRP    # 8 n-groups
NWARM = 6          # warmup matmuls: spans PE-ready (~8 us) to data-ready (~10.7)

# test.py hooks: set TRACE=True before calling kernel() to profile; the
# BassKernelResults of the last run lands in LAST_RESULTS.
TRACE = os.environ.get("BASS_KERNEL_TRACE", "0") == "1"
LAST_RESULTS = None

_cached_nc = None


def _build():
    nc = bacc.Bacc(
        "TRN2", target_bir_lowering=False, debug=False, enable_asserts=False, num_devices=N_CORES
    )
    f32 = mybir.dt.float32
    f16 = mybir.dt.float16
    lhsT = nc.dram_tensor("lhsT", [MT, P, KT * 128], f16, kind="ExternalInput")
    rhs = nc.dram_tensor("rhs", [D, N2], f16, kind="ExternalInput")
    biasv = nc.dram_tensor("biasv", [P, 1], f32, kind="ExternalInput")
    out = nc.dram_tensor("out", [M, N2], f16, kind="ExternalOutput")

    # K-major DRAM view split into [P, KT, cols]: rhs_r[p, kt, n] is
    # rhs row kt*128+p, matching the per-k-tile partition layout.
    rhs_r = rhs[:, :].rearrange("(kt p) n -> p kt n", p=P)
    # Partition-first view of the packed lhsT for multi-m-tile DMAs.
    lhsT_r = lhsT[:, :, :].rearrange("m p j -> p m j")

    with TileContext(nc) as tc:
        with (
            tc.tile_pool(name="const", bufs=1) as constp,
            tc.tile_pool(name="lhs", bufs=1) as lhsp,
            tc.tile_pool(name="rhsp", bufs=4) as rhsp,
            tc.tile_pool(name="outp", bufs=8) as outp,
            tc.tile_pool(name="psum", bufs=4, space="PSUM") as psump,
        ):
            # Warmup tiles zeroed on GpSimd, whose queue is ready first
            # after the preamble; the warm matmuls then keep the PE busy
            # through the HAM activity window while the head DMAs land.
            warm_w = constp.tile([P, P], f16, tag="warmw")
            warm_r = constp.tile([P, NFREE], f16, tag="warmr")
            nc.gpsimd.memset(warm_w[:], 0.0)
            nc.gpsimd.memset(warm_r[:], 0.0)

            lt = lhsp.tile([P, MT, KT * 128], f16, tag="lhs")
            rt0 = rhsp.tile([P, KT, NGRP], f16, tag="rhs")
            bias_t = constp.tile([P, 1], f32, tag="bias")

            # Priority-ordered head loads. Each dma_start costs the
            # issuing engine ~0.6-0.8 us of DIRECT2D descriptor time,
            # so pieces are consolidated: the whole-DMA completion
            # semaphore is the availability granule anyway. Need order
            # at the cold 427 ns/MM rate: h0 k0-k3 over the first 4
            # real matmuls, then h1 k0-k3, then lt m1..7 one tile per
            # 1.7 us.
            nc.sync.dma_start(
                out=rt0[:, 0:2, 0:NFREE], in_=rhs_r[:, 0:2, 0:NFREE]
            )
            nc.scalar.dma_start(out=bias_t[:], in_=biasv[:, :])
            nc.scalar.dma_start(out=lt[:, 0, :], in_=lhsT[0, :, :])
            nc.sync.dma_start(
                out=rt0[:, 2:4, 0:NFREE], in_=rhs_r[:, 2:4, 0:NFREE]
            )
            nc.sync.dma_start(
                out=rt0[:, 0:2, NFREE:NGRP], in_=rhs_r[:, 0:2, NFREE:NGRP]
            )
            nc.sync.dma_start(
                out=rt0[:, 2:4, NFREE:NGRP], in_=rhs_r[:, 2:4, NFREE:NGRP]
            )
            nc.scalar.dma_start(out=lt[:, 1:4, :], in_=lhsT_r[:, 1:4, :])
            nc.gpsimd.dma_start(out=lt[:, 4:MT, :], in_=lhsT_r[:, 4:MT, :])

            # Warm up the PE's HAM clock gate during the head-load
            # window: dummy matmuls (no data deps beyond the memsets)
            # push the PE through its ~3.4 us busy window so the real
            # matmuls run at 2.4 GHz almost immediately.
            wps = psump.tile([P, NFREE], f32, tag="ps1")
            for i in range(NWARM):
                nc.tensor.matmul(
                    wps[:], warm_w[:], warm_r[:],
                    start=(i == 0), stop=(i == NWARM - 1),
                )

            # Steady-state rhs loads run on the GpSimd (SWDGE) queue so
            # they never sit behind output stores in the HWDGE FIFOs.
            rts = {0: rt0}

            def load_rhs(g):
                rt = rhsp.tile([P, KT, NGRP], f16, tag="rhs")
                nc.gpsimd.dma_start(
                    out=rt[:], in_=rhs_r[:, :, g * NGRP : (g + 1) * NGRP]
                )
                rts[g] = rt

            for n in range(NT):
                rt = rts.pop(n)
                for m in range(MT):
                    # Stagger rhs prefetch, keeping up to two groups of
                    # lookahead: iter 0 loads groups 1 and 2 (staggered),
                    # iter n>=1 tops up with group n+2.
                    if m == 0:
                        if n == 0:
                            load_rhs(1)
                        elif n + 2 < NT:
                            load_rhs(n + 2)
                    if m == 4 and n == 0:
                        load_rhs(2)
                    last = n == NT - 1 and m == MT - 1
                    ps0 = psump.tile([P, NFREE], f32, tag="ps0")
                    ps1 = psump.tile([P, NFREE], f32, tag="ps1")
                    w = lambda k: lt[:, m, k * P : (k + 1) * P]
                    for k in range(KT):
                        nc.tensor.matmul(
                            ps0[:], w(k), rt[:, k, 0:NFREE],
                            start=(k == 0), stop=(k == KT - 1),
                        )
                    ot = outp.tile([P, NGRP], f16, tag="ot")
                    # ps0's copy is emitted before ps1's k-loop so ACT
                    # overlaps the ps1 matmuls (different PSUM banks).
                    nc.scalar.activation(
                        ot[:, 0:NFREE], ps0[:],
                        mybir.ActivationFunctionType.Identity,
                        bias=bias_t[:, 0:1],
                    )
                    for k in range(KT):
                        nc.tensor.matmul(
                            ps1[:], w(k), rt[:, k, NFREE:NGRP],
                            start=(k == 0), stop=(k == KT - 1),
                        )
                    if last:
                        # Final tile: split ps1's copy between ACT and
                        # DVE and the stores over the two HWDGE rings
                        # (SWDGE has multi-us issue+drain latency — never
                        # put the critical final store there) so the
                        # kernel-exit barrier waits on ~0.5 us of work
                        # instead of a serial copy+store chain.
                        nc.sync.dma_start(
                            out=out[m * P : (m + 1) * P,
                                    n * NGRP : n * NGRP + NFREE],
                            in_=ot[:, 0:NFREE],
                        )
                        nc.scalar.activation(
                            ot[:, NFREE : NFREE + 256], ps1[:, 0:256],
                            mybir.ActivationFunctionType.Identity,
                            bias=bias_t[:, 0:1],
                        )
                        nc.vector.tensor_scalar_add(
                            ot[:, NFREE + 256 : NGRP], ps1[:, 256:NFREE],
                            bias_t[:, 0:1],
                        )
                        nc.scalar.dma_start(
                            out=out[m * P : (m + 1) * P,
                                    n * NGRP + NFREE : n * NGRP + NFREE + 256],
                            in_=ot[:, NFREE : NFREE + 256],
                        )
                        nc.sync.dma_start(
                            out=out[m * P : (m + 1) * P,
                                    n * NGRP + NFREE + 256 : (n + 1) * NGRP],
                            in_=ot[:, NFREE + 256 : NGRP],
                        )
                    else:
                        nc.vector.tensor_scalar_add(
                            ot[:, NFREE:NGRP], ps1[:], bias_t[:, 0:1]
                        )
                        # Alternate stores across the two HWDGE rings so
                        # the store stream drains on both.
                        st = nc.sync if m % 2 == 0 else nc.scalar
                        st.dma_start(
                            out=out[m * P : (m + 1) * P,
                                    n * NGRP : (n + 1) * NGRP],
                            in_=ot[:],
                        )
    nc.compile()
    return nc


def kernel(input1, input2, weight, bias, type_index):
    global _cached_nc, LAST_RESULTS

    input1 = np.asarray(input1, dtype=np.float32)
    input2 = np.asarray(input2, dtype=np.float32)
    weight = np.asarray(weight, dtype=np.float32)
    bias = np.asarray(bias, dtype=np.float32).reshape(-1)
    w_r = weight[int(type_index)]  # [D]

    # Host-side prep: fold the w_r row-scale into input1, lay both GEMM
    # operands out K-major, cast to fp16 (device accumulates in fp32).
    scaled = input1 * w_r[None, :]  # [N1, D]
    rhsT = np.ascontiguousarray(input2.T.astype(np.float16))  # [D, N2]
    bias_vec = np.full((P, 1), float(bias[0]), dtype=np.float32)

    in_maps = []
    for c in range(N_CORES):
        shard = scaled[c * M : (c + 1) * M]  # [M, D]
        # Pack per-m-tile weight blocks: lhsT[m, p, k*128+j] =
        # shard[m*128+j, k*128+p], so each m-tile is one contiguous DMA
        # and each k slice is a [K=128, M=128] stationary operand.
        a = shard.T.astype(np.float16).reshape(KT, P, MT, P)
        lhsT_packed = np.ascontiguousarray(
            a.transpose(2, 1, 0, 3).reshape(MT, P, KT * P)
        )
        in_maps.append(
            {
                "lhsT": lhsT_packed,
                "rhs": rhsT,
                "biasv": bias_vec,
            }
        )

    if _cached_nc is None:
        _cached_nc = _build()

    res = run_bass_kernel_spmd(
        _cached_nc, in_maps, core_ids=list(range(N_CORES)), trace=TRACE
    )
    LAST_RESULTS = res
    return np.concatenate(
        [res.results[c]["out"] for c in range(N_CORES)], axis=0
    ).astype(np.float32)


# revision 9
# speedup vs baseline: 1.0513x; 1.0513x over previous
"""DistMult decoder kernel for 8 Trainium2 NeuronCores.

Computes out = (input1 * weight[type_index]) @ input2.T + bias with
input1 [8192, 512], input2 [8192, 512] in fp32, out [8192, 8192].

Sharding: rows of input1 (and thus rows of the output) are split across
the 8 cores; input2 / weight / bias are replicated. No communication.

Per-core device program (M = 1024 rows):
  - lhsT  [MT, P, KT*128] = w_r-scaled shard of input1, packed on host
    into per-m-tile blocks (one contiguous 128 KB DMA per m-tile)
  - rhs   [512, 8192]  = input2 transposed + cast to fp16 on host
  - fp16 operands run the PE at 1 cycle/row with fp32 PSUM accumulation
  - GEMM over n-pairs: 8 groups x 8 m-tiles x (4 k x 2 n) matmuls,
    215.8 ns/matmul steady state (the N=512 fp16 streaming floor)
  - output stored as fp16 (16 MB/core instead of 32 MB) and upcast on
    host; total quantization error ~8e-4 vs the 2e-2 gate

Head/tail schedule (from baseline trace analysis: body exec starts
~6.1 us after a fixed preamble, first DMA data lands ~3 us later):
  - 8 warmup matmuls on zeroed SBUF start at body-exec time and span
    the ~3.4 us HAM busy window, so the PE clock is at/near 2.4 GHz
    when the first real matmul's data arrives (~9.5 us)
  - head DMAs are priority-ordered across the three DGE rings: the
    first n-group quarters and first m-tiles land before later ones
  - last tile: ps0 copy overlaps ps1's matmuls, ps1 copy is split
    ACT/DVE, and the final stores fan out over all three rings
"""

import os

import numpy as np

import concourse.bacc as bacc
import concourse.mybir as mybir
from concourse.bass_utils import run_bass_kernel_spmd
from concourse.tile import TileContext

N_CORES = 8
N1, N2, D = 8192, 8192, 512
M = N1 // N_CORES  # rows per core
P = 128            # partitions
KT = D // P        # 4 k-tiles
MT = M // P        # 8 m-tiles
NFREE = 512        # psum bank free size (fp32)
NGRP = 1024        # n columns per group (pair of psum banks)
NT = N2 // NGRP    # 8 n-groups
NWARM = 11         # warmup matmuls: spans PE-ready (~7.6 us) to rt0-landed (~12.3)

# test.py hooks: set TRACE=True before calling kernel() to profile; the
# BassKernelResults of the last run lands in LAST_RESULTS.
TRACE = os.environ.get("BASS_KERNEL_TRACE", "0") == "1"
LAST_RESULTS = None

_cached_nc = None


def _build():
    nc = bacc.Bacc(
        "TRN2", target_bir_lowering=False, debug=False, enable_asserts=False, num_devices=N_CORES
    )
    f32 = mybir.dt.float32
    f16 = mybir.dt.float16
    lhsT = nc.dram_tensor("lhsT", [MT, P, KT * 128], f16, kind="ExternalInput")
    rhs = nc.dram_tensor("rhs", [D, N2], f16, kind="ExternalInput")
    biasv = nc.dram_tensor("biasv", [P, 1], f32, kind="ExternalInput")
    out = nc.dram_tensor("out", [M, N2], f16, kind="ExternalOutput")

    # K-major DRAM view split into [P, KT, cols]: rhs_r[p, kt, n] is
    # rhs row kt*128+p, matching the per-k-tile partition layout.
    rhs_r = rhs[:, :].rearrange("(kt p) n -> p kt n", p=P)
    # Partition-first view of the packed lhsT for multi-m-tile DMAs.
    lhsT_r = lhsT[:, :, :].rearrange("m p j -> p m j")

    with TileContext(nc) as tc:
        with (
            tc.tile_pool(name="const", bufs=1) as constp,
            tc.tile_pool(name="lhs", bufs=1) as lhsp,
            tc.tile_pool(name="rhsp", bufs=4) as rhsp,
            tc.tile_pool(name="outp", bufs=8) as outp,
            tc.tile_pool(name="psum", bufs=4, space="PSUM") as psump,
        ):
            # Warmup tiles zeroed on GpSimd, whose queue is ready first
            # after the preamble; the warm matmuls then keep the PE busy
            # through the HAM activity window while the head DMAs land.
            warm_w = constp.tile([P, P], f16, tag="warmw")
            warm_r = constp.tile([P, NFREE], f16, tag="warmr")
            nc.gpsimd.memset(warm_w[:], 0.0)
            nc.gpsimd.memset(warm_r[:], 0.0)

            lt = lhsp.tile([P, MT, KT * 128], f16, tag="lhs")
            rt0 = rhsp.tile([P, KT, NGRP], f16, tag="rhs")
            bias_t = constp.tile([P, 1], f32, tag="bias")

            # Priority-ordered head loads, byte-balanced across the
            # three DGE rings (each ring sustains only ~110 GB/s; the
            # warm MM stream consumes rt0 at ~590 GB/s, so real MMs
            # start only once rt0 has fully landed, ~12.3 us). The 8
            # rt0 k/half pieces (128 KB each) are striped round-robin
            # sync/scalar/gpsimd in k-loop consumption order, with the
            # lt m-tiles interleaved by their later need times.
            def rq(eng, k, h):
                eng.dma_start(
                    out=rt0[:, k, h * NFREE : (h + 1) * NFREE],
                    in_=rhs_r[:, k, h * NFREE : (h + 1) * NFREE],
                )

            nc.scalar.dma_start(out=bias_t[:], in_=biasv[:, :])
            rq(nc.sync, 0, 0)
            nc.scalar.dma_start(out=lt[:, 0, :], in_=lhsT[0, :, :])
            rq(nc.gpsimd, 2, 0)
            rq(nc.sync, 3, 0)
            rq(nc.scalar, 1, 0)
            rq(nc.gpsimd, 1, 1)
            rq(nc.sync, 2, 1)
            rq(nc.scalar, 0, 1)
            rq(nc.gpsimd, 3, 1)
            nc.scalar.dma_start(out=lt[:, 1, :], in_=lhsT[1, :, :])
            nc.gpsimd.dma_start(out=lt[:, 2, :], in_=lhsT[2, :, :])
            nc.sync.dma_start(out=lt[:, 3, :], in_=lhsT[3, :, :])
            nc.scalar.dma_start(out=lt[:, 4, :], in_=lhsT[4, :, :])
            nc.gpsimd.dma_start(out=lt[:, 5, :], in_=lhsT[5, :, :])
            nc.sync.dma_start(out=lt[:, 6, :], in_=lhsT[6, :, :])
            nc.scalar.dma_start(out=lt[:, 7, :], in_=lhsT[7, :, :])

            # Warm up the PE's HAM clock gate during the head-load
            # window: dummy matmuls (no data deps beyond the memsets)
            # push the PE through its ~3.4 us busy window so the real
            # matmuls run at 2.4 GHz almost immediately.
            wps = psump.tile([P, NFREE], f32, tag="ps1")
            for i in range(NWARM):
                nc.tensor.matmul(
                    wps[:], warm_w[:], warm_r[:],
                    start=(i == 0), stop=(i == NWARM - 1),
                )

            # Steady-state rhs loads run on the GpSimd (SWDGE) queue so
            # they never sit behind output stores in the HWDGE FIFOs.
            rts = {0: rt0}

            def load_rhs(g):
                rt = rhsp.tile([P, KT, NGRP], f16, tag="rhs")
                nc.gpsimd.dma_start(
                    out=rt[:], in_=rhs_r[:, :, g * NGRP : (g + 1) * NGRP]
                )
                rts[g] = rt

            for n in range(NT):
                rt = rts.pop(n)
                for m in range(MT):
                    # Stagger rhs prefetch, keeping up to two groups of
                    # lookahead: iter 0 loads groups 1 and 2 (staggered),
                    # iter n>=1 tops up with group n+2.
                    if m == 0:
                        if n == 0:
                            load_rhs(1)
                        elif n + 2 < NT:
                            load_rhs(n + 2)
                    if m == 4 and n == 0:
                        load_rhs(2)
                    last = n == NT - 1 and m == MT - 1
                    ps0 = psump.tile([P, NFREE], f32, tag="ps0")
                    ps1 = psump.tile([P, NFREE], f32, tag="ps1")
                    w = lambda k: lt[:, m, k * P : (k + 1) * P]
                    for k in range(KT):
                        nc.tensor.matmul(
                            ps0[:], w(k), rt[:, k, 0:NFREE],
                            start=(k == 0), stop=(k == KT - 1),
                        )
                    ot = outp.tile([P, NGRP], f16, tag="ot")
                    # ps0's copy is emitted before ps1's k-loop so ACT
                    # overlaps the ps1 matmuls (different PSUM banks).
                    nc.scalar.activation(
                        ot[:, 0:NFREE], ps0[:],
                        mybir.ActivationFunctionType.Identity,
                        bias=bias_t[:, 0:1],
                    )
                    for k in range(KT):
                        nc.tensor.matmul(
                            ps1[:], w(k), rt[:, k, NFREE:NGRP],
                            start=(k == 0), stop=(k == KT - 1),
                        )
                    if last:
                        # Final tile: split ps1's copy between ACT and
                        # DVE and the stores over the two HWDGE rings
                        # (SWDGE has multi-us issue+drain latency — never
                        # put the critical final store there) so the
                        # kernel-exit barrier waits on ~0.5 us of work
                        # instead of a serial copy+store chain.
                        nc.sync.dma_start(
                            out=out[m * P : (m + 1) * P,
                                    n * NGRP : n * NGRP + NFREE],
                            in_=ot[:, 0:NFREE],
                        )
                        nc.scalar.activation(
                            ot[:, NFREE : NFREE + 256], ps1[:, 0:256],
                            mybir.ActivationFunctionType.Identity,
                            bias=bias_t[:, 0:1],
                        )
                        nc.vector.tensor_scalar_add(
                            ot[:, NFREE + 256 : NGRP], ps1[:, 256:NFREE],
                            bias_t[:, 0:1],
                        )
                        nc.scalar.dma_start(
                            out=out[m * P : (m + 1) * P,
                                    n * NGRP + NFREE : n * NGRP + NFREE + 256],
                            in_=ot[:, NFREE : NFREE + 256],
                        )
                        nc.sync.dma_start(
                            out=out[m * P : (m + 1) * P,
                                    n * NGRP + NFREE + 256 : (n + 1) * NGRP],
                            in_=ot[:, NFREE + 256 : NGRP],
                        )
                    else:
                        nc.vector.tensor_scalar_add(
                            ot[:, NFREE:NGRP], ps1[:], bias_t[:, 0:1]
                        )
                        # Alternate stores across the two HWDGE rings so
                        # the store stream drains on both.
                        st = nc.sync if m % 2 == 0 else nc.scalar
                        st.dma_start(
                            out=out[m * P : (m + 1) * P,
                                    n * NGRP : (n + 1) * NGRP],
                            in_=ot[:],
                        )
    nc.compile()
    return nc


def kernel(input1, input2, weight, bias, type_index):
    global _cached_nc, LAST_RESULTS

    input1 = np.asarray(input1, dtype=np.float32)
    input2 = np.asarray(input2, dtype=np.float32)
    weight = np.asarray(weight, dtype=np.float32)
    bias = np.asarray(bias, dtype=np.float32).reshape(-1)
    w_r = weight[int(type_index)]  # [D]

    # Host-side prep: fold the w_r row-scale into input1, lay both GEMM
    # operands out K-major, cast to fp16 (device accumulates in fp32).
    scaled = input1 * w_r[None, :]  # [N1, D]
    rhsT = np.ascontiguousarray(input2.T.astype(np.float16))  # [D, N2]
    bias_vec = np.full((P, 1), float(bias[0]), dtype=np.float32)

    in_maps = []
    for c in range(N_CORES):
        shard = scaled[c * M : (c + 1) * M]  # [M, D]
        # Pack per-m-tile weight blocks: lhsT[m, p, k*128+j] =
        # shard[m*128+j, k*128+p], so each m-tile is one contiguous DMA
        # and each k slice is a [K=128, M=128] stationary operand.
        a = shard.T.astype(np.float16).reshape(KT, P, MT, P)
        lhsT_packed = np.ascontiguousarray(
            a.transpose(2, 1, 0, 3).reshape(MT, P, KT * P)
        )
        in_maps.append(
            {
                "lhsT": lhsT_packed,
                "rhs": rhsT,
                "biasv": bias_vec,
            }
        )

    if _cached_nc is None:
        _cached_nc = _build()

    res = run_bass_kernel_spmd(
        _cached_nc, in_maps, core_ids=list(range(N_CORES)), trace=TRACE
    )
    LAST_RESULTS = res
    return np.concatenate(
        [res.results[c]["out"] for c in range(N_CORES)], axis=0
    ).astype(np.float32)


# revision 10
# speedup vs baseline: 1.0900x; 1.0368x over previous
"""DistMult decoder kernel for 8 Trainium2 NeuronCores.

Computes out = (input1 * weight[type_index]) @ input2.T + bias with
input1 [8192, 512], input2 [8192, 512] in fp32, out [8192, 8192].

Sharding: rows of input1 (and thus rows of the output) are split across
the 8 cores; input2 / weight / bias are replicated. No communication.

Per-core device program (M = 1024 rows):
  - lhsT  [MT, P, KT*128] = w_r-scaled shard of input1, packed on host
    into per-m-tile blocks (one contiguous 128 KB DMA per m-tile)
  - rhs   [512, 8192]  = input2 transposed + cast to fp16 on host
  - fp16 operands run the PE at 1 cycle/row with fp32 PSUM accumulation
  - GEMM over 16 n-groups of 512 cols x 8 m-tiles x 4 k matmuls;
    215.8 ns/matmul steady state (the N=512 fp16 streaming floor)
  - output stored as fp16 (16 MB/core instead of 32) and upcast on the
    host; total quantization error ~5e-4 vs the 2e-2 gate

Schedule rationale (from trace analysis): ~6 us fixed preamble; first
DMA data lands ~8.7 us; during the head window all 8 cores load
simultaneously so per-core aggregate is only ~180-300 GB/s. 512-col
n-groups keep the critical first-matmul set small (512 KB rhs + 128 KB
lhsT), spread round-robin over the three DGE rings in consumption
order. Warmup matmuls on zeroed SBUF keep the PE busy from ~7.6 us so
the HAM clock gate opens (2.4 GHz) before the real stream starts.
"""

import os

import numpy as np

import concourse.bacc as bacc
import concourse.mybir as mybir
from concourse.bass_utils import run_bass_kernel_spmd
from concourse.tile import TileContext

N_CORES = 8
N1, N2, D = 8192, 8192, 512
M = N1 // N_CORES  # rows per core
P = 128            # partitions
KT = D // P        # 4 k-tiles
MT = M // P        # 8 m-tiles
NG = 512           # n columns per group (one psum bank)
NT = N2 // NG      # 16 n-groups
NWARM = 10         # warmup matmuls: spans PE-ready (~7.6 us) to data-ready (~12)

TRACE = os.environ.get("BASS_KERNEL_TRACE", "0") == "1"
LAST_RESULTS = None

_cached_nc = None


def _build():
    nc = bacc.Bacc(
        "TRN2", target_bir_lowering=False, debug=False, enable_asserts=False, num_devices=N_CORES
    )
    f32 = mybir.dt.float32
    f16 = mybir.dt.float16
    lhsT = nc.dram_tensor("lhsT", [MT, P, KT * P], f16, kind="ExternalInput")
    rhs = nc.dram_tensor("rhs", [D, N2], f16, kind="ExternalInput")
    biasv = nc.dram_tensor("biasv", [P, 1], f32, kind="ExternalInput")
    out = nc.dram_tensor("out", [M, N2], f16, kind="ExternalOutput")

    # K-major DRAM view split into [P, KT, cols]: rhs_r[p, kt, n] is
    # rhs row kt*128+p, matching the per-k-tile partition layout.
    rhs_r = rhs[:, :].rearrange("(kt p) n -> p kt n", p=P)

    with TileContext(nc) as tc:
        with (
            tc.tile_pool(name="const", bufs=1) as constp,
            tc.tile_pool(name="lhs", bufs=1) as lhsp,
            tc.tile_pool(name="rhsp", bufs=4) as rhsp,
            tc.tile_pool(name="outp", bufs=8) as outp,
            tc.tile_pool(name="psum", bufs=4, space="PSUM") as psump,
        ):
            # Warmup tiles zeroed on GpSimd (ready first after preamble).
            warm_w = constp.tile([P, P], f16, tag="warmw")
            warm_r = constp.tile([P, NG], f16, tag="warmr")
            nc.gpsimd.memset(warm_w[:], 0.0)
            nc.gpsimd.memset(warm_r[:], 0.0)

            lt = lhsp.tile([P, MT, KT * P], f16, tag="lhs")
            bias_t = constp.tile([P, 1], f32, tag="bias")
            rts = {}

            def rtile(g):
                rt = rhsp.tile([P, KT, NG], f16, tag="rhs")
                rts[g] = rt
                return rt

            rt0, rt1 = rtile(0), rtile(1)

            # Priority-ordered head loads, round-robin across the three
            # DGE rings in consumption order (~128 KB pieces). The real
            # stream needs rt0 + lt m0 first; g1's quarters and the
            # later m-tiles interleave by their deadlines.
            nc.scalar.dma_start(out=bias_t[:], in_=biasv[:, :])
            nc.sync.dma_start(out=rt0[:, 0, :], in_=rhs_r[:, 0, 0:NG])
            nc.scalar.dma_start(out=lt[:, 0, :], in_=lhsT[0, :, :])
            nc.gpsimd.dma_start(out=rt0[:, 1, :], in_=rhs_r[:, 1, 0:NG])
            nc.sync.dma_start(out=rt0[:, 2, :], in_=rhs_r[:, 2, 0:NG])
            nc.scalar.dma_start(out=rt0[:, 3, :], in_=rhs_r[:, 3, 0:NG])
            nc.gpsimd.dma_start(out=lt[:, 1, :], in_=lhsT[1, :, :])
            nc.sync.dma_start(out=lt[:, 2, :], in_=lhsT[2, :, :])
            nc.scalar.dma_start(out=lt[:, 3, :], in_=lhsT[3, :, :])
            nc.gpsimd.dma_start(out=lt[:, 4, :], in_=lhsT[4, :, :])
            nc.sync.dma_start(out=rt1[:, 0, :], in_=rhs_r[:, 0, NG : 2 * NG])
            nc.scalar.dma_start(out=lt[:, 5, :], in_=lhsT[5, :, :])
            nc.gpsimd.dma_start(out=rt1[:, 1, :], in_=rhs_r[:, 1, NG : 2 * NG])
            nc.sync.dma_start(out=lt[:, 6, :], in_=lhsT[6, :, :])
            nc.scalar.dma_start(out=rt1[:, 2, :], in_=rhs_r[:, 2, NG : 2 * NG])
            nc.gpsimd.dma_start(out=lt[:, 7, :], in_=lhsT[7, :, :])
            nc.sync.dma_start(out=rt1[:, 3, :], in_=rhs_r[:, 3, NG : 2 * NG])

            # Warm up the PE's HAM clock gate during the head-load
            # window so the real matmuls start at 2.4 GHz.
            wps = psump.tile([P, NG], f32, tag="ps")
            for i in range(NWARM):
                nc.tensor.matmul(
                    wps[:], warm_w[:], warm_r[:],
                    start=(i == 0), stop=(i == NWARM - 1),
                )

            # Steady-state rhs prefetch on the GpSimd (SWDGE) queue:
            # latency-tolerant, never behind the HWDGE store streams.
            def load_rhs(g):
                rt = rtile(g)
                nc.gpsimd.dma_start(
                    out=rt[:], in_=rhs_r[:, :, g * NG : (g + 1) * NG]
                )

            for g in range(NT):
                rt = rts.pop(g)
                for m in range(MT):
                    if m == 0 and 2 <= g + 2 < NT:
                        load_rhs(g + 2)
                    last = g == NT - 1 and m == MT - 1
                    ps = psump.tile([P, NG], f32, tag="ps")
                    for k in range(KT):
                        nc.tensor.matmul(
                            ps[:], lt[:, m, k * P : (k + 1) * P],
                            rt[:, k, :],
                            start=(k == 0), stop=(k == KT - 1),
                        )
                    ot = outp.tile([P, NG], f16, tag="ot")
                    if last:
                        # Final tile: split the copy between ACT and DVE
                        # and the store over both HWDGE rings so the
                        # exit barrier waits on minimal serial work.
                        nc.scalar.activation(
                            ot[:, 0:256], ps[:, 0:256],
                            mybir.ActivationFunctionType.Identity,
                            bias=bias_t[:, 0:1],
                        )
                        nc.vector.tensor_scalar_add(
                            ot[:, 256:NG], ps[:, 256:NG], bias_t[:, 0:1]
                        )
                        nc.sync.dma_start(
                            out=out[m * P : (m + 1) * P,
                                    g * NG : g * NG + 256],
                            in_=ot[:, 0:256],
                        )
                        nc.scalar.dma_start(
                            out=out[m * P : (m + 1) * P,
                                    g * NG + 256 : (g + 1) * NG],
                            in_=ot[:, 256:NG],
                        )
                    else:
                        # Alternate psum->sbuf+bias copies between ACT
                        # and DVE, and stores between the HWDGE rings.
                        if m % 2 == 0:
                            nc.scalar.activation(
                                ot[:], ps[:],
                                mybir.ActivationFunctionType.Identity,
                                bias=bias_t[:, 0:1],
                            )
                        else:
                            nc.vector.tensor_scalar_add(
                                ot[:], ps[:], bias_t[:, 0:1]
                            )
                        st = nc.sync if m % 2 == 0 else nc.scalar
                        st.dma_start(
                            out=out[m * P : (m + 1) * P,
                                    g * NG : (g + 1) * NG],
                            in_=ot[:],
                        )
    nc.compile()
    return nc


def kernel(input1, input2, weight, bias, type_index):
    global _cached_nc, LAST_RESULTS

    input1 = np.asarray(input1, dtype=np.float32)
    input2 = np.asarray(input2, dtype=np.float32)
    weight = np.asarray(weight, dtype=np.float32)
    bias = np.asarray(bias, dtype=np.float32).reshape(-1)
    w_r = weight[int(type_index)]  # [D]

    # Host-side prep: fold the w_r row-scale into input1, lay both GEMM
    # operands out K-major, cast to fp16 (device accumulates in fp32).
    scaled = input1 * w_r[None, :]  # [N1, D]
    rhsT = np.ascontiguousarray(input2.T.astype(np.float16))  # [D, N2]
    bias_vec = np.full((P, 1), float(bias[0]), dtype=np.float32)

    in_maps = []
    for c in range(N_CORES):
        shard = scaled[c * M : (c + 1) * M]  # [M, D]
        # Pack per-m-tile weight blocks: lhsT[m, p, k*128+j] =
        # shard[m*128+j, k*128+p], so each m-tile is one contiguous DMA
        # and each k slice is a [K=128, M=128] stationary operand.
        a = shard.T.astype(np.float16).reshape(KT, P, MT, P)
        lhsT_packed = np.ascontiguousarray(
            a.transpose(2, 1, 0, 3).reshape(MT, P, KT * P)
        )
        in_maps.append(
            {
                "lhsT": lhsT_packed,
                "rhs": rhsT,
                "biasv": bias_vec,
            }
        )

    if _cached_nc is None:
        _cached_nc = _build()

    res = run_bass_kernel_spmd(
        _cached_nc, in_maps, core_ids=list(range(N_CORES)), trace=TRACE
    )
    LAST_RESULTS = res
    return np.concatenate(
        [res.results[c]["out"] for c in range(N_CORES)], axis=0
    ).astype(np.float32)
